# revision 40
# baseline (speedup 1.0000x reference)
"""BiasedMultiHeadAttention Trainium2 kernel (v5: attention-shadowed projections).

Sharding: 8 cores = (batch b, query-half qh). Each core computes the full
pipeline for its 512 query rows of batch b (K/V projections for the batch
are duplicated across the 2 cores sharing it). No collectives.

Device layout trick: per-core x rows are host-rolled so the core's query
block is always rows 0..511 -> one SPMD program for all 8 cores; bias/mask
are rolled consistently (softmax sum order irrelevant).

Math folding (host, exact):
  LN affine folded into weights; Q scaled by SCALE. The per-head gated bias
  is pre-exponentiated on host: eb = exp(gate_h * bias); the device computes
  softmax numerators as exp(logits) * eb (one vector multiply, no bias add).

Structure: the softmax exp on the Scalar engine (64 x ~1.1us) is the
irreducible critical path. Q/K/V projections (fp8 DoubleRow, 2x PE rate)
are streamed as PE filler inside the attention window so the whole kernel
rides the exp cadence: per head-pair t the c-loop runs QK -> exp -> *eb ->
AV while interleaving the projection matmuls for head-pair t+1. O-proj
(bf16) accumulates during the last pair and finishes after it.

PSUM budget (8 banks): QK ps [128,2,512] x2 bufs = 4, AV avA/avB [128,512]
= 2, projections pp [128,512] x2 bufs = 2. V is padded to 128 columns
(64 ch | ones | zeros) so AV stationaries are full 128-wide (FWL).
"""

import numpy as np
import ml_dtypes

import concourse.bass as bass
import concourse.tile as tile
import concourse.mybir as mybir
from concourse import bacc
from concourse.bass_utils import run_bass_kernel_spmd
from concourse.masks import make_identity

B, L, E, H = 4, 1024, 1024, 16
D = E // H
SCALE = D**-0.5
EPS = 1e-5
NCORES = 8
QL = 512  # query rows per core
PT = 128  # partitions
NL = L // PT  # 8 l-chunks
NE = E // PT  # 8 e-chunks
HP = H // 2  # 8 head pairs

F32 = mybir.dt.float32
BF16 = mybir.dt.bfloat16
FP8 = mybir.dt.float8e4
I32 = mybir.dt.int32
BF_NP = ml_dtypes.bfloat16
FP8_NP = ml_dtypes.float8_e4m3
# host-side up-scales keeping fp8 weights ~N(0, 0.5); undone at psum evac
QSC, KSC, VSC = 64.0, 16.0, 16.0

LAST_RESULT = None  # BassKernelResults of the most recent run (for test.py)


def _build_nc(use_pbias, use_mask, dump=()):
    """Build the single-core Bass program (same NEFF for all 8 cores)."""
    nc = bacc.Bacc("TRN2", target_bir_lowering=False, debug=False)
    dump_d = {}

    def dump_tile(name, ap):
        if name in dump:
            d = nc.dram_tensor("d_" + name, list(ap.shape), ap.dtype,
                               kind="ExternalOutput")
            nc.sync.dma_start(d[tuple(slice(None) for _ in ap.shape)], ap)
            dump_d[name] = d

    x_d = nc.dram_tensor("xc", [PT, NL, L], F32, kind="ExternalInput")
    eb_d = nc.dram_tensor("ebc", [H, L, QL], BF16, kind="ExternalInput")
    wq_d = nc.dram_tensor("wqt", [PT, NE, E], FP8, kind="ExternalInput")
    wk_d = nc.dram_tensor("wkt", [PT, NE, E], FP8, kind="ExternalInput")
    wv_d = nc.dram_tensor("wvt", [PT, NE, E], FP8, kind="ExternalInput")
    wo_d = nc.dram_tensor("wot", [PT, NE, E], BF16, kind="ExternalInput")
    pb_d = {}
    for name, use in zip("qkvo", use_pbias):
        if use:
            pb_d[name] = nc.dram_tensor(f"b{name}e", [1, E], BF16,
                                        kind="ExternalInput")
    if use_mask:
        km_d = nc.dram_tensor("kmc", [PT, NL], F32, kind="ExternalInput")
        mq_d = nc.dram_tensor("mqc", [1, QL], F32, kind="ExternalInput")
    y_d = nc.dram_tensor("yc", [QL, E], F32, kind="ExternalOutput")

    DR = mybir.MatmulPerfMode.DoubleRow
    NK2 = NE // 2  # fp8 DoubleRow contracts pairs of 128-chunks

    with tile.TileContext(nc) as tc:
        with (
            tc.tile_pool(name="persist", bufs=1) as pp,
            tc.tile_pool(name="consts", bufs=1) as cp,
        ):
            # ---- constants ----
            ident = cp.tile([PT, PT], BF16)
            make_identity(nc, ident)
            if any(use_pbias):
                ones_row = cp.tile([1, L], BF16)
                nc.vector.memset(ones_row, 1.0)
            eps_t = cp.tile([PT, 1], F32)
            nc.vector.memset(eps_t, EPS)
            if use_mask:
                km_sb = cp.tile([PT, NL], F32)
                nc.sync.dma_start(km_sb, km_d[:, :])
                mqb = cp.tile([64, QL], F32)
                nc.gpsimd.dma_start(mqb,
                                    mq_d[0:1, :].partition_broadcast(64))

            # ---- resident tensors ----
            x_sb = pp.tile([PT, NL, L], F32)
            for lt in range(NL):
                nc.sync.dma_start(x_sb[:, lt, :], x_d[:, lt, :])
            wo_sb = pp.tile([PT, NE, E], BF16)
            # K^T zero-padded per head parity (full-128 QK contracts with
            # the other head's rows zeroed; keeps FWL on the weight path)
            kTzA = pp.tile([PT, NE, L], BF16)
            kTzB = pp.tile([PT, NE, L], BF16)
            # V padded to 128 cols per head: [ch(64) | ones | junk(63)].
            # av rows 65:128 are never read, so the junk cols stay
            # uninitialized instead of paying a big gpsimd memset.
            v4 = pp.tile([PT, NL, H, PT], BF16)
            nc.vector.memset(v4[:, :, :, 64:65], 1.0)
            qT = pp.tile([PT, NE, QL], BF16)    # Q^T (scaled) [e_q, q]
            oT = pp.tile([PT, NE, QL], BF16)    # attnout^T (normalized)
            pbr = {}
            for name in pb_d:
                pbr[name] = cp.tile([1, E], BF16)
                nc.sync.dma_start(pbr[name], pb_d[name][:, :])

            with (
                tc.tile_pool(name="xnt", bufs=1) as xp,
                tc.tile_pool(name="wt", bufs=1) as wtp,
            ):
                # weights go via the gpsimd DMA ring so they don't queue
                # behind x/eb on the sync ring; wv first (needed earliest),
                # wo last (needed only by O-proj). kTz memsets go after the
                # DMA issues so they don't delay them.
                xnT = xp.tile([PT, NE, L], FP8)  # xn^T [e, l]
                wv_sb = wtp.tile([PT, NE, E], FP8)
                nc.gpsimd.dma_start(wv_sb, wv_d[:, :, :])
                wk_sb = wtp.tile([PT, NE, E], FP8)
                nc.gpsimd.dma_start(wk_sb, wk_d[:, :, :])
                wq_sb = wtp.tile([PT, NE, E], FP8)
                nc.gpsimd.dma_start(wq_sb, wq_d[:, :, :])
                nc.gpsimd.dma_start(wo_sb, wo_d[:, :, :])
                nc.gpsimd.memset(kTzA[64:128, :, :], 0.0)
                nc.gpsimd.memset(kTzB[0:64, :, :], 0.0)

                with (
                    tc.tile_pool(name="prj", bufs=2, space="PSUM") as prp,
                ):
                    # ---------- projection helpers (filler units) --------
                    def proj_k(ot):
                        osl = slice(ot * PT, (ot + 1) * PT)
                        for nh in range(2):
                            nsl = slice(nh * 512, (nh + 1) * 512)
                            ps = prp.tile([PT, 512], F32, tag="pp",
                                          name=f"psk{ot}{nh}")
                            for k2 in range(NK2):
                                ksl = slice(2 * k2, 2 * k2 + 2)
                                yield nc.tensor.matmul(
                                    ps, wk_sb[:, ksl, osl], xnT[:, ksl, nsl],
                                    start=(k2 == 0),
                                    stop=(k2 == NK2 - 1 and "k" not in pbr),
                                    perf_mode=DR)
                            if "k" in pbr:
                                yield nc.tensor.matmul(
                                    ps, pbr["k"][:, osl], ones_row[:, 0:512],
                                    start=False, stop=True)
                            yield nc.scalar.mul(kTzA[0:64, ot, nsl],
                                                ps[0:64, :], 1.0 / KSC)
                            yield nc.vector.tensor_scalar_mul(
                                kTzB[64:128, ot, nsl], ps[64:128, :],
                                1.0 / KSC)

                    def proj_q(ot):
                        osl = slice(ot * PT, (ot + 1) * PT)
                        psq = prp.tile([PT, 512], F32, tag="pp",
                                       name=f"psq{ot}")
                        for k2 in range(NK2):
                            ksl = slice(2 * k2, 2 * k2 + 2)
                            yield nc.tensor.matmul(
                                psq, wq_sb[:, ksl, osl], xnT[:, ksl, 0:512],
                                start=(k2 == 0),
                                stop=(k2 == NK2 - 1 and "q" not in pbr),
                                perf_mode=DR)
                        if "q" in pbr:
                            yield nc.tensor.matmul(
                                psq, pbr["q"][:, osl], ones_row[:, 0:512],
                                start=False, stop=True)
                        yield nc.vector.tensor_scalar_mul(qT[:, ot, 0:512],
                                                          psq, 1.0 / QSC)

                    def proj_v(lt, vh):
                        lsl = slice(lt * PT, (lt + 1) * PT)
                        vsl = slice(vh * 512, (vh + 1) * 512)
                        psv = prp.tile([PT, 512], F32, tag="pp",
                                       name=f"psv{lt}{vh}")
                        for k2 in range(NK2):
                            ksl = slice(2 * k2, 2 * k2 + 2)
                            yield nc.tensor.matmul(
                                psv, xnT[:, ksl, lsl], wv_sb[:, ksl, vsl],
                                start=(k2 == 0),
                                stop=(k2 == NK2 - 1 and "v" not in pbr),
                                perf_mode=DR)
                        if "v" in pbr:
                            yield nc.tensor.matmul(
                                psv, ones_row[:, 0:PT], pbr["v"][:, vsl],
                                start=False, stop=True)
                        yield nc.scalar.mul(
                            v4[:, lt, vh * 8:(vh + 1) * 8, 0:64],
                            psv.rearrange("p (h d) -> p h d", h=8),
                            1.0 / VSC)

                    # ========== Phase 1: LayerNorm + transpose + V(vh0) ==
                    lp = tc.alloc_tile_pool(name="ln", bufs=3)
                    ptp = tc.alloc_tile_pool(name="pst", bufs=2,
                                             space="PSUM")
                    for lt in range(NL):
                        xr = x_sb[:, lt, :].rearrange("p (s d) -> p s d",
                                                      s=2)
                        stats = lp.tile([PT, 2, 6], F32, tag="stats")
                        for sg in range(2):
                            nc.vector.bn_stats(stats[:, sg, :], xr[:, sg, :])
                        mv = lp.tile([PT, 2], F32, tag="mv")
                        nc.vector.bn_aggr(mv, stats)
                        sd = lp.tile([PT, 1], F32, tag="sd")
                        nc.scalar.activation(
                            sd, mv[:, 1:2],
                            mybir.ActivationFunctionType.Sqrt, bias=eps_t)
                        rs = lp.tile([PT, 1], F32, tag="rs")
                        nc.vector.reciprocal(rs, sd)
                        nmr = lp.tile([PT, 1], F32, tag="nmr")  # -mu*rs
                        nc.vector.tensor_scalar(
                            out=nmr, in0=mv[:, 0:1], scalar1=rs,
                            scalar2=-1.0, op0=mybir.AluOpType.mult,
                            op1=mybir.AluOpType.mult)
                        xnb = lp.tile([PT, L], BF16, tag="xnb")
                        nc.scalar.activation(
                            xnb, x_sb[:, lt, :],
                            mybir.ActivationFunctionType.Identity,
                            bias=nmr, scale=rs)
                        for g in range(2):
                            psT = ptp.tile([PT, 512], BF16, tag="psT")
                            for j in range(4):
                                et = g * 4 + j
                                nc.tensor.transpose(
                                    psT[:, j * PT:(j + 1) * PT],
                                    xnb[:, et * PT:(et + 1) * PT], ident)
                            dst = xnT[:, g * 4:(g + 1) * 4,
                                      lt * PT:(lt + 1) * PT]
                            src = psT.rearrange("p (j l) -> p j l", j=4)
                            if g == 0:
                                nc.scalar.copy(dst, src)
                            else:
                                nc.vector.tensor_copy(dst, src)
                        # V(lt, vh=0) right behind this lt's transposes
                        for _ in proj_v(lt, 0):
                            pass

                    ptp.release()
                    lp.release()
                    scp = tc.alloc_tile_pool(name="sc", bufs=2,
                                             space="PSUM")
                    avp = tc.alloc_tile_pool(name="av", bufs=1,
                                             space="PSUM")
                    bp = tc.alloc_tile_pool(name="bias", bufs=4)
                    sp = tc.alloc_tile_pool(name="expp", bufs=3)
                    ap = tc.alloc_tile_pool(name="attn", bufs=3)
                    rcp = tc.alloc_tile_pool(name="nrm", bufs=2)
                    oop = tc.alloc_tile_pool(name="oo", bufs=2)
                    yop = tc.alloc_tile_pool(name="yo", bufs=2)

                    dump_tile("xnT", xnT[:, :, :])

                    # K/Q for head pair 0 (rest streams in the shadow)
                    for _ in proj_k(0):
                        pass
                    for _ in proj_q(0):
                        pass

                    # filler stream: K/Q for pairs 1..7, V vh=1 for all lt
                    def filler_ops():
                        for t1 in range(1, HP):
                            yield from proj_k(t1)
                            yield from proj_q(t1)
                            if t1 <= 4:
                                for lt in (2 * (t1 - 1), 2 * t1 - 1):
                                    yield from proj_v(lt, 1)
                    filler = filler_ops()
                    FILLER_PER_C = 4

                    # ================= attention ======================
                    pending = []  # deferred normalize ops (prev pair)
                    for t in range(HP):
                        hA, hB = 2 * t, 2 * t + 1
                        avA = avp.tile([PT, QL], F32, tag="avA")
                        avB = avp.tile([PT, QL], F32, tag="avB")

                        # QK issued one chunk ahead so the scalar exp chain
                        # never queues behind AV/filler matmuls (PE is
                        # in-order).
                        ps_tiles = {}

                        def issue_qk(c, t=t):
                            csl = slice(c * PT, (c + 1) * PT)
                            ps = scp.tile([PT, 2, QL], F32, tag="ps",
                                          name=f"ps{t}{c}")
                            nc.tensor.matmul(ps[:, 0, :], kTzA[:, t, csl],
                                             qT[:, t, :], start=True,
                                             stop=True)
                            nc.tensor.matmul(ps[:, 1, :], kTzB[:, t, csl],
                                             qT[:, t, :], start=True,
                                             stop=True)
                            ps_tiles[c] = ps

                        issue_qk(0)
                        for c in range(NL):
                            csl = slice(c * PT, (c + 1) * PT)
                            ps = ps_tiles.pop(c)
                            ebt = bp.tile([PT, 2, QL], BF16, tag="ebt")
                            nc.sync.dma_start(
                                ebt, eb_d[hA:hB + 1, csl, :].rearrange(
                                    "h p q -> p h q"))
                            es = sp.tile([PT, 2, QL], BF16, tag="es")
                            kmb = km_sb[:, c:c + 1] if use_mask else 0.0
                            nc.scalar.activation(
                                es.rearrange("p h q -> p (h q)"),
                                ps.rearrange("p h q -> p (h q)"),
                                mybir.ActivationFunctionType.Exp, bias=kmb)
                            if c + 1 < NL:
                                issue_qk(c + 1)
                            at = ap.tile([PT, 2, QL], BF16, tag="at")
                            nc.vector.tensor_mul(
                                at.rearrange("p h q -> p (h q)"),
                                es.rearrange("p h q -> p (h q)"),
                                ebt.rearrange("p h q -> p (h q)"))
                            if t == 0 and c == 0:
                                dump_tile("at0", at[:, 0, :])
                                dump_tile("at1", at[:, 1, :])
                            st, sp_ = (c == 0), (c == NL - 1)
                            nc.tensor.matmul(avA, v4[:, c, hA, :],
                                             at[:, 0, :], start=st,
                                             stop=sp_)
                            nc.tensor.matmul(avB, v4[:, c, hB, :],
                                             at[:, 1, :], start=st,
                                             stop=sp_)
                            for _ in range(FILLER_PER_C):
                                if next(filler, None) is None:
                                    break
                            if pending and c >= 1:
                                pending.pop(0)()
                        # ---- normalize part 1: evacuate av banks to
                        # SBUF right away so the next pair's AVs can
                        # claim them; recip/muls run deferred inside the
                        # next pair's stream (keeps the in-order vector
                        # queue from stalling on the gpsimd broadcast).
                        rsb = rcp.tile([65, 2, QL], F32, tag="rsb")
                        nc.vector.tensor_copy(rsb[0:65, 0, :], avA[0:65, :])
                        nc.vector.tensor_copy(rsb[0:65, 1, :], avB[0:65, :])
                        rsrow = rcp.tile([1, 2 * QL], F32, tag="rsrow",
                                         bufs=1)
                        nc.gpsimd.dma_start(
                            rsrow,
                            rsb[64:65, :, :].rearrange("p h q -> p (h q)"))
                        rbs = oop.tile([64, 2, QL], F32, tag="rbs")
                        nc.gpsimd.partition_broadcast(
                            rbs.rearrange("p h q -> p (h q)"), rsrow[0:1, :])

                        def make_norm(t, rsb, rbs):
                            def do_recip():
                                nc.vector.reciprocal_approx_fast(
                                    out=rbs.rearrange("p h q -> p (h q)"),
                                    in_=rbs.rearrange("p h q -> p (h q)"))
                                if use_mask:
                                    for hi in range(2):
                                        nc.vector.tensor_mul(
                                            rbs[:, hi, :], rbs[:, hi, :],
                                            mqb)
                            def do_muls():
                                nc.vector.tensor_mul(oT[0:64, t, :],
                                                     rsb[0:64, 0, :],
                                                     rbs[:, 0, :])
                                ot_odd = oop.tile([64, QL], BF16, tag="oo")
                                nc.vector.tensor_mul(ot_odd,
                                                     rsb[0:64, 1, :],
                                                     rbs[:, 1, :])
                                nc.sync.dma_start(oT[64:128, t, :], ot_odd)
                            return [do_recip, do_muls]

                        pending.extend(make_norm(t, rsb, rbs))
                    while pending:
                        pending.pop(0)()

                    dump_tile("oT", oT[:, :, :])

                    # ====== O-proj in [q, e] + residual ======
                    for qb in range(4):
                        qsl = slice(qb * PT, (qb + 1) * PT)
                        for eh in range(2):
                            esl = slice(eh * 512, (eh + 1) * 512)
                            psf = prp.tile([PT, 512], F32, tag="pp",
                                           name=f"psf{qb}{eh}")
                            for ic in range(NE):
                                nc.tensor.matmul(
                                    psf, oT[:, ic, qsl], wo_sb[:, ic, esl],
                                    start=(ic == 0),
                                    stop=(ic == NE - 1 and "o" not in pbr))
                            if "o" in pbr:
                                nc.tensor.matmul(psf, ones_row[0:1, 0:PT],
                                                 pbr["o"][:, esl],
                                                 start=False, stop=True)
                            y_sb = yop.tile([PT, 512], F32, tag="y")
                            nc.vector.tensor_add(y_sb, psf,
                                                 x_sb[:, qb, esl])
                            nc.sync.dma_start(y_d[qsl, esl], y_sb)
                    for _pool in (yop, oop, rcp, ap, sp, bp, avp, scp):
                        _pool.release()
    return nc


def _prep_inputs(x, bias, mask, wq, bq, wk, bk, wv, bv, wo, bo, gate,
                 ln_g, ln_b):
    """Host-side folding + per-core sharding. Returns (in_maps, meta)."""
    gate = np.asarray(gate, np.float32)
    ln_g = np.asarray(ln_g, np.float32)
    ln_b = np.asarray(ln_b, np.float32)

    wqt = (np.asarray(wq).T * ln_g[:, None] * (SCALE * QSC)).astype(FP8_NP)
    wkt = (np.asarray(wk).T * ln_g[:, None] * KSC).astype(FP8_NP)
    wvt = (np.asarray(wv).T * ln_g[:, None] * VSC).astype(FP8_NP)
    wot = np.asarray(wo).T.astype(BF_NP)
    bqe = ((np.asarray(wq) @ ln_b + np.asarray(bq))
           * (SCALE * QSC)).astype(np.float32)
    bke = ((np.asarray(wk) @ ln_b + np.asarray(bk)) * KSC).astype(np.float32)
    bve = ((np.asarray(wv) @ ln_b + np.asarray(bv)) * VSC).astype(np.float32)
    boe = np.asarray(bo, np.float32)
    use_pbias = tuple(bool(np.any(b)) for b in (bqe, bke, bve, boe))

    mask = np.asarray(mask, np.int32)
    use_mask = not bool(np.all(mask == 1))

    def wfmt(w):  # [E_in, E_out] -> [128, 8, E]
        return np.ascontiguousarray(
            w.reshape(NE, PT, E).transpose(1, 0, 2))

    shared = {"wqt": wfmt(wqt), "wkt": wfmt(wkt), "wvt": wfmt(wvt),
              "wot": wfmt(wot)}
    for name, use, b in zip("qkvo", use_pbias, (bqe, bke, bve, boe)):
        if use:
            shared[f"b{name}e"] = b.reshape(1, E).astype(BF_NP)

    x = np.asarray(x, np.float32)
    bias = np.asarray(bias, np.float32)
    in_maps = []
    for c in range(NCORES):
        b_idx, qh = divmod(c, 2)
        q0 = qh * QL
        xr = np.roll(x[b_idx], -q0, axis=0)  # query block first
        m = {}
        m.update(shared)
        m["xc"] = np.ascontiguousarray(
            xr.reshape(NL, PT, L).transpose(1, 0, 2))
        bs = bias[b_idx][:, q0:q0 + QL, :]  # [H, QL, L]
        bs = np.roll(bs, -q0, axis=2)       # roll key axis
        eb = np.exp(gate[:, None, None] * bs)
        m["ebc"] = np.ascontiguousarray(eb.swapaxes(1, 2)).astype(BF_NP)
        if use_mask:
            mr = np.roll(mask[b_idx], -q0)
            kmf = (-10000.0 * (1.0 - mr.astype(np.float32)))
            m["kmc"] = np.ascontiguousarray(
                kmf.reshape(NL, PT).T).astype(np.float32)
            m["mqc"] = mr[:QL].astype(np.float32).reshape(1, QL)
        in_maps.append(m)
    return in_maps, (use_pbias, use_mask)


def kernel(**inputs):
    global LAST_RESULT
    in_maps, (use_pbias, use_mask) = _prep_inputs(**inputs)
    nc = _build_nc(use_pbias, use_mask)
    if not nc.is_finalized():
        nc.finalize()
    res = run_bass_kernel_spmd(nc, in_maps, core_ids=list(range(NCORES)))
    LAST_RESULT = res
    out = np.empty((B, L, E), np.float32)
    for c in range(NCORES):
        b_idx, qh = divmod(c, 2)
        out[b_idx, qh * QL:(qh + 1) * QL, :] = res.results[c]["yc"]
    return out
